# revision 42
# baseline (speedup 1.0000x reference)
"""Trainium2 Bass kernel for one BLT transformer layer (B=2, S=2048, D=2048,
H=16, KVH=4, HD=128, I=8192, fp32 I/O).

Sharding v2: token-parallel with K/V AllGather and causally-balanced chunk
pairing. Core c handles batch b=c//4 and the two 256-token chunks (j, 7-j)
with j=c%4, i.e. 512 "own" tokens laid out [shallow chunk | deep chunk].
Each core projects Q/K/V only for its own tokens, AllGathers K and V across
its 4-core batch group (two HBM collectives, overlapped with the Q
projection), and runs attention against the full 2048-key set with causal
block skipping: the shallow chunk (global position < 1024) only attends to
key blocks 0..7 (kc 0..7, N=512 matmuls cover both chunks), the deep chunk
additionally attends to kc 8..15 (N=256 matmuls). Causality inside the
computed region is a host-supplied binary mask multiplied into exp(scores).
The O-projection, both RMSNorms, and the full MLP run on the 512 own tokens
with no redundancy, so per-core matmul work is ~4.2k [128x128x512]-
equivalents vs ~5k for the dense sequence-parallel layout.

On-chip layout is feature-major [feature, token]; matmuls run in fp16/bf16
with fp32 PSUM accumulation; softmax/norm math in fp32. RoPE's interleaved
pairs become contiguous halves via a host-side even/odd permutation of the
wq/wk rows. Softmax skips max-subtraction (scores bounded here). exp() is
evicted from two-bank [128,1024] PSUM tiles to halve ACT instruction
overhead; softmax denominators are computed by DVE tensor_reduce over the
key-block axis plus one fp32 partition-sum matmul per head.
"""

import os
from contextlib import ExitStack

import ml_dtypes
import numpy as np

import concourse.bacc as bacc
import concourse.mybir as mybir
import concourse.tile as tile
from concourse.bass_utils import run_bass_kernel_spmd
from concourse.masks import make_identity

F16 = mybir.dt.float16
BF16 = mybir.dt.bfloat16
F32 = mybir.dt.float32
AF = mybir.ActivationFunctionType
OP = mybir.AluOpType
AX = mybir.AxisListType

P = 128
EPS = 1e-6
NEG_THRESH = -0.5  # additive mask values are 0.0 or -1e9
SCALE = 128.0 ** -0.5

D, S, H, KVH, I = 2048, 2048, 16, 4, 8192
TQ = 512            # own tokens per core
DC = D // P         # 16 d-model chunks
IT = I // P         # 64 intermediate tiles
KC = S // P         # 16 key chunks of the full batch
GROUPS = [[0, 1, 2, 3], [4, 5, 6, 7]]

# Whether to AllGather K/V across the batch group (2 HBM collectives) or
# compute K/V for the full batch redundantly on every core. Collectives
# save ~450 matmul-equivalents/core but (observed) cap the PE clock at
# 13/16 for the whole NEFF, so the dense-local variant is the default.
USE_CC = bool(int(os.environ.get("KERNEL_USE_CC", "0")))

LAST_EXEC_NS = None


# --------------------------------------------------------------------------
# kernel body (built once per process)
# --------------------------------------------------------------------------

def build_nc(debug=False):
    nc = bacc.Bacc("TRN2", target_bir_lowering=False, debug=debug)

    t = {}
    t["xq16"] = nc.dram_tensor("xq16", [D, TQ], F16, kind="ExternalInput")
    t["xq32"] = nc.dram_tensor("xq32", [D, TQ], F32, kind="ExternalInput")
    t["cos_q"] = nc.dram_tensor("cos_q", [64, TQ], F32, kind="ExternalInput")
    t["sin_q"] = nc.dram_tensor("sin_q", [64, TQ], F32, kind="ExternalInput")
    if USE_CC:
        t["cos_k"] = nc.dram_tensor("cos_k", [64, TQ], F32, kind="ExternalInput")
        t["sin_k"] = nc.dram_tensor("sin_k", [64, TQ], F32, kind="ExternalInput")
    else:
        t["xkv"] = nc.dram_tensor("xkv", [D, S], F16, kind="ExternalInput")
        t["cos_k"] = nc.dram_tensor("cos_k", [64, S], F32, kind="ExternalInput")
        t["sin_k"] = nc.dram_tensor("sin_k", [64, S], F32, kind="ExternalInput")
    t["mask1"] = nc.dram_tensor("mask1", [P, 8, 256], BF16, kind="ExternalInput")
    t["mask2"] = nc.dram_tensor("mask2", [P, 8, 256], BF16, kind="ExternalInput")
    t["wq_t"] = nc.dram_tensor("wq_t", [H, P, DC, P], F16, kind="ExternalInput")
    if USE_CC:
        t["wk_t"] = nc.dram_tensor("wk_t", [KVH, P, DC, P], F16,
                                   kind="ExternalInput")
    else:
        t["wk_t2"] = nc.dram_tensor("wk_t2", [DC, P, KVH, P], F16,
                                    kind="ExternalInput")
    t["wv_r"] = nc.dram_tensor("wv_r", [DC, P, KVH * P], F16, kind="ExternalInput")
    t["wo_t"] = nc.dram_tensor("wo_t", [DC, P, H, P], F16, kind="ExternalInput")
    t["wg_t"] = nc.dram_tensor("wg_t", [IT, P, DC, P], F16, kind="ExternalInput")
    t["wu_t"] = nc.dram_tensor("wu_t", [IT, P, DC, P], F16, kind="ExternalInput")
    t["wd_t"] = nc.dram_tensor("wd_t", [DC, P, IT, P], F16, kind="ExternalInput")
    t["outT"] = nc.dram_tensor("outT", [D, TQ], F32, kind="ExternalOutput")

    with tile.TileContext(nc) as tc:
        _body(nc, tc, t)
    nc.compile()
    return nc


def _body(nc, tc, t):
    with ExitStack() as ctx:
        misc = ctx.enter_context(tc.tile_pool(name="misc", bufs=1, side="right"))
        dram = ctx.enter_context(tc.tile_pool(name="dram", bufs=1, space="DRAM"))

        ones16 = misc.tile([P, 1], F16, tag="ones16")
        nc.vector.memset(ones16[:], 1.0)
        ones_bf = misc.tile([P, 1], BF16, tag="ones_bf")
        nc.vector.memset(ones_bf[:], 1.0)
        ones32 = misc.tile([1, P], F32, tag="ones32")
        nc.vector.memset(ones32[:], 1.0)
        onescol = misc.tile([P, 1], F32, tag="onescol")
        nc.vector.memset(onescol[:], 1.0)
        ident = misc.tile([P, P], F32, tag="ident")
        make_identity(nc, ident[:])

        if USE_CC:
            cc_k_in = dram.tile([P, KVH, TQ], F16, name="cc_k_in")
            cc_k_out = dram.tile([4, P, KVH, TQ], F16, name="cc_k_out")
            cc_v_in = dram.tile([P, 4, 512], BF16, name="cc_v_in")
            cc_v_out = dram.tile([4, P, 4, 512], BF16, name="cc_v_out")

        def recip(out_ap, in_ap):
            sc = misc.tile([1, 512], F32, tag="rscratch", bufs=1, name="rsc")
            nc.vector.reciprocal_approx_accurate(
                out_ap, in_ap, sc[:, :out_ap.shape[-1]])

        def rstd_from_var(var_ps, d_dim):
            """psum var-sum [1,N] -> sbuf rstd [1,N] fp32."""
            r = misc.tile([1, var_ps.shape[-1]], F32, tag="rstd_tmp", bufs=2)
            nc.vector.tensor_scalar(
                r[:], var_ps[:], 1.0 / d_dim, EPS, OP.mult, OP.add
            )
            recip(r[:], r[:])
            nc.scalar.activation(r[:], r[:], AF.Sqrt)
            return r

        # persistent tensors; allocation order sets left-stack release order:
        # p_att2 (attnT, dies after o-proj) below p_att (K/V/Q, dies after
        # attention) below p0b (hq/cos, dies after attention).
        p_att2 = tc.alloc_tile_pool(name="p_att2", bufs=1, side="left")
        attnT = p_att2.tile([P, H, TQ], F16, tag="attnT")
        p_att = tc.alloc_tile_pool(name="p_att", bufs=1, side="left")
        KT = p_att.tile([P, KVH, S], F16, tag="KT")
        Vt = p_att.tile([P, KC, 512], BF16, tag="Vt")
        QT = p_att.tile([P, H, TQ], F16, tag="QT")
        p0b = tc.alloc_tile_pool(name="p0b", bufs=1, side="left")
        hq = p0b.tile([P, DC, TQ], F16, tag="hq")
        cosq = p0b.tile([64, TQ], F32, tag="cosq")
        sinq = p0b.tile([64, TQ], F32, tag="sinq")
        rdbq = p0b.tile([P, TQ], F32, tag="rdbq")

        # ================= phase 0+1: norm + K/V projections ===============
        wq_stage = {}
        with tc.tile_pool(name="p0", bufs=1, side="left") as p0, \
                tc.tile_pool(name="s0", bufs=1, side="left") as s0, \
                tc.tile_pool(name="pp1", bufs=1, space="PSUM") as pp1:
            TK = TQ if USE_CC else S
            cosk = p0.tile([64, TK], F32, tag="cosk")
            sink = p0.tile([64, TK], F32, tag="sink")
            if USE_CC:
                for dc in range(DC):
                    nc.scalar.dma_start(hq[:, dc, :],
                                        t["xq16"][dc * P:(dc + 1) * P, :])
                nc.scalar.dma_start(cosq[:], t["cos_q"][:])
                nc.scalar.dma_start(sinq[:], t["sin_q"][:])
                nc.scalar.dma_start(cosk[:], t["cos_k"][:])
                nc.scalar.dma_start(sink[:], t["sin_k"][:])

            def bcast(row_ap, out_sb):
                pbc = pp1.tile([P, 512], F32, tag="bc", bufs=1, name="pbc0")
                n = row_ap.shape[-1]
                nc.tensor.matmul(pbc[:, :n], ones32[:], row_ap,
                                 start=True, stop=True)
                nc.scalar.activation(out_sb, pbc[:, :n], AF.Copy)

            def rstd_cols(rdb_ap, col_ap):
                """transpose 128-col slices of a broadcast tile into
                per-token partition-indexed columns."""
                for c in range(rdb_ap.shape[-1] // P):
                    tp = pp1.tile([P, 512], F32, tag="bc", bufs=1, name="tp")
                    nc.tensor.transpose(tp[:, :P],
                                        rdb_ap[:, c * P:(c + 1) * P], ident[:])
                    nc.scalar.activation(col_ap[:, c:c + 1], tp[:, 0:1],
                                         AF.Copy)

            def rope(ps, cos_ap, sin_ap, out_ap):
                """ps [128,N] psum fp32 (rows 0:64 = re, 64:128 = im,
                permuted), out_ap [128,N] fp16."""
                n = cos_ap.shape[-1]
                re, im = ps[0:64, :], ps[64:128, :]
                t1 = s0.tile([64, TQ], F32, tag="rope1", bufs=2)
                t2 = s0.tile([64, TQ], F32, tag="rope2", bufs=2)
                nc.vector.tensor_tensor(t1[:, :n], re, cos_ap, OP.mult)
                nc.vector.tensor_tensor(t2[:, :n], im, sin_ap, OP.mult)
                nc.vector.tensor_tensor(out_ap[0:64, :], t1[:, :n], t2[:, :n],
                                        OP.subtract)
                nc.vector.tensor_tensor(t1[:, :n], re, sin_ap, OP.mult)
                nc.vector.tensor_tensor(t2[:, :n], im, cos_ap, OP.mult)
                nc.vector.tensor_tensor(out_ap[64:128, :], t1[:, :n], t2[:, :n],
                                        OP.add)

            def own_var():
                # own-token variance -> rdbq (for the Q rope folding)
                varq = pp1.tile([1, 512], F32, tag="small", bufs=1, name="varq")
                for dc in range(DC):
                    sq = s0.tile([P, TQ], F16, tag="sq", bufs=3)
                    nc.scalar.activation(sq[:], hq[:, dc, :], AF.Square)
                    nc.tensor.matmul(varq[:, :TQ], ones16[:], sq[:],
                                     start=(dc == 0), stop=(dc == DC - 1))
                rq = rstd_from_var(varq[:, :TQ], D)
                bcast(rq[:], rdbq[:])
                nc.vector.tensor_tensor(cosq[:], cosq[:], rdbq[:64, :], OP.mult)
                nc.vector.tensor_tensor(sinq[:], sinq[:], rdbq[:64, :], OP.mult)

            if USE_CC:
                own_var()
                nc.vector.tensor_tensor(cosk[:], cosk[:], rdbq[:64, :], OP.mult)
                nc.vector.tensor_tensor(sink[:], sink[:], rdbq[:64, :], OP.mult)
                rstd_col = p0.tile([P, 4], F32, tag="rstd_col")
                rstd_cols(rdbq[:], rstd_col[:])

                # ---- K projection + rope -> AllGather ----
                K_own = p0.tile([P, KVH, TQ], F16, tag="K_own")
                for et in range(KVH):
                    wk_sb = s0.tile([P, DC, P], F16, tag="wkq", bufs=3,
                                    name="wk_sb")
                    nc.sync.dma_start(wk_sb[:], t["wk_t"][et])
                    pk = pp1.tile([P, 512], F32, tag="pj", bufs=6, name="pk")
                    for dc in range(DC):
                        nc.tensor.matmul(pk[:, :TQ], wk_sb[:, dc, :],
                                         hq[:, dc, :],
                                         start=(dc == 0), stop=(dc == DC - 1))
                    rope(pk[:, :TQ], cosk[:], sink[:], K_own[:, et, :])
                nc.gpsimd.dma_start(cc_k_in[:], K_own[:])
                nc.gpsimd.collective_compute(
                    "AllGather", OP.bypass, replica_groups=GROUPS,
                    ins=[cc_k_in.opt()], outs=[cc_k_out.opt()],
                )
                for r in range(4):
                    nc.gpsimd.dma_start(KT[:, :, 256 * r:256 * (r + 1)],
                                        cc_k_out[r, :, :, 0:256])
                    nc.gpsimd.dma_start(KT[:, :, 256 * (7 - r):256 * (8 - r)],
                                        cc_k_out[r, :, :, 256:512])

                # ---- V projection (rstd folded at eviction) -> AllGather ----
                wv_sb = p0.tile([P, DC, KVH * P], F16, tag="wv")
                for dc in range(DC):
                    nc.sync.dma_start(wv_sb[:, dc, :], t["wv_r"][dc])
                V_own = p0.tile([P, 4, 512], BF16, tag="V_own")
                for c in range(4):
                    pv = pp1.tile([P, 512], F32, tag="pj", bufs=6, name="pv")
                    for dc in range(DC):
                        nc.tensor.matmul(
                            pv[:], hq[:, dc, c * P:(c + 1) * P], wv_sb[:, dc, :],
                            start=(dc == 0), stop=(dc == DC - 1),
                        )
                    nc.scalar.activation(V_own[:, c, :], pv[:], AF.Copy,
                                         scale=rstd_col[:, c:c + 1])
                nc.gpsimd.dma_start(cc_v_in[:], V_own[:])
                nc.gpsimd.collective_compute(
                    "AllGather", OP.bypass, replica_groups=GROUPS,
                    ins=[cc_v_in.opt()], outs=[cc_v_out.opt()],
                )
                for r in range(4):
                    for c, kc in ((0, 2 * r), (1, 2 * r + 1),
                                  (2, 14 - 2 * r), (3, 15 - 2 * r)):
                        nc.gpsimd.dma_start(Vt[:, kc, :], cc_v_out[r, :, c, :])
            else:
                # ---- dense local K/V over the full batch, 512-col groups ----
                # DMA queues: sync = even x chunks, gpsimd = odd x chunks
                # then wv; scalar = wk (dc-major, matching the per-dc
                # interleaved consumption), cos, then hq.
                wk_all = p0.tile([P, DC, KVH, P], F16, tag="wk_all")
                for dc in range(DC):
                    nc.scalar.dma_start(wk_all[:, dc, :, :], t["wk_t2"][dc])
                nc.scalar.dma_start(cosk[:], t["cos_k"][:])
                nc.scalar.dma_start(sink[:], t["sin_k"][:])
                wv_sb = p0.tile([P, DC, KVH * P], F16, tag="wv")
                rstd_col = p0.tile([P, KC], F32, tag="rstd_col")
                for g in range(4):
                    gs = slice(g * 512, (g + 1) * 512)
                    hn = s0.tile([P, DC, 512], F16, tag="hn", bufs=2, name="hn")
                    for dc in range(DC):
                        eng = (nc.sync, nc.gpsimd)[dc % 2]
                        eng.dma_start(
                            hn[:, dc, :], t["xkv"][dc * P:(dc + 1) * P, gs])
                    if g == 0:
                        # wv behind group 0's odd chunks on the gpsimd queue;
                        # hq/cos behind group 0's even chunks on sync (keeps
                        # the scalar queue free for the rstd/eviction ACTs)
                        for dc in range(DC):
                            nc.gpsimd.dma_start(wv_sb[:, dc, :], t["wv_r"][dc])
                    if g == 1:
                        nc.sync.dma_start(cosq[:], t["cos_q"][:])
                        nc.sync.dma_start(sinq[:], t["sin_q"][:])
                        for dc in range(DC):
                            eng = (nc.sync, nc.gpsimd)[dc % 2]
                            eng.dma_start(hq[:, dc, :],
                                          t["xq16"][dc * P:(dc + 1) * P, :])
                    # variance + 4 K-head chains interleaved per dc so the PE
                    # rides right behind the x stream (5 live PSUM chains)
                    var_g = pp1.tile([1, 512], F32, tag="small", bufs=1,
                                     name="var_g")
                    pks = [pp1.tile([P, 512], F32, tag="pj", bufs=6, name="pk")
                           for _ in range(KVH)]
                    for dc in range(DC):
                        sq = s0.tile([P, 512], F16, tag="sq", bufs=3)
                        nc.scalar.activation(sq[:], hn[:, dc, :], AF.Square)
                        nc.tensor.matmul(var_g[:], ones16[:], sq[:],
                                         start=(dc == 0), stop=(dc == DC - 1))
                        for et in range(KVH):
                            nc.tensor.matmul(pks[et][:],
                                             wk_all[:, dc, et, :],
                                             hn[:, dc, :],
                                             start=(dc == 0),
                                             stop=(dc == DC - 1))
                    rg = rstd_from_var(var_g[:], D)
                    rdbg = s0.tile([P, 512], F32, tag="rdbg", bufs=2)
                    bcast(rg[:], rdbg[:])
                    rstd_cols(rdbg[:], rstd_col[:, 4 * g:4 * (g + 1)])
                    nc.vector.tensor_tensor(cosk[:, gs], cosk[:, gs],
                                            rdbg[:64, :], OP.mult)
                    nc.vector.tensor_tensor(sink[:, gs], sink[:, gs],
                                            rdbg[:64, :], OP.mult)
                    for et in range(KVH):
                        rope(pks[et][:], cosk[:, gs], sink[:, gs], KT[:, et, gs])
                    for c in range(4):
                        kc = 4 * g + c
                        pv = pp1.tile([P, 512], F32, tag="pj", bufs=6, name="pv")
                        for dc in range(DC):
                            nc.tensor.matmul(
                                pv[:], hn[:, dc, c * P:(c + 1) * P],
                                wv_sb[:, dc, :],
                                start=(dc == 0), stop=(dc == DC - 1),
                            )
                        nc.scalar.activation(Vt[:, kc, :], pv[:], AF.Copy,
                                             scale=rstd_col[:, kc:kc + 1])
                    if g == 2:
                        # own-token norm early enough that rdbq/cos are ready
                        # for the first Q ropes right at attention start
                        own_var()
                        for hh in (0, 1):
                            w = p0b.tile([P, DC, P], F16, tag="wq01",
                                         name="wq01")
                            nc.sync.dma_start(w[:], t["wq_t"][hh])
                            wq_stage[hh] = w

        # ========= phase 2: attention with Q-proj software-pipelined =======
        # Per iteration h: finish(h-2) [bcast+normalize], AV(h-1), Q(h+2)
        # projection+rope, scores(h) with the den chain of h-1 interleaved
        # between score groups (fills the PE while exp(h) evictions lag).
        n_rep = H // KVH
        with tc.tile_pool(name="s2", bufs=1, side="left") as s2, \
                tc.tile_pool(name="pp2", bufs=1, space="PSUM") as pp2:
            mk1 = s2.tile([P, 8, 256], BF16, tag="mk1")
            mk2 = s2.tile([P, 8, 256], BF16, tag="mk2")
            nc.sync.dma_start(mk1[:], t["mask1"][:])
            nc.sync.dma_start(mk2[:], t["mask2"][:])

            def rope2(ps, cos_ap, sin_ap, out_ap):
                re, im = ps[0:64, :], ps[64:128, :]
                t1 = s2.tile([64, TQ], F32, tag="rope1", bufs=2)
                t2 = s2.tile([64, TQ], F32, tag="rope2", bufs=2)
                nc.vector.tensor_tensor(t1[:], re, cos_ap, OP.mult)
                nc.vector.tensor_tensor(t2[:], im, sin_ap, OP.mult)
                nc.vector.tensor_tensor(out_ap[0:64, :], t1[:], t2[:],
                                        OP.subtract)
                nc.vector.tensor_tensor(t1[:], re, sin_ap, OP.mult)
                nc.vector.tensor_tensor(t2[:], im, cos_ap, OP.mult)
                nc.vector.tensor_tensor(out_ap[64:128, :], t1[:], t2[:], OP.add)

            wq_tiles = dict(wq_stage)

            def wq_fetch(h):
                if h >= H or h in wq_tiles:
                    return
                w = s2.tile([P, DC, P], F16, tag="wkq", bufs=4, name="wq_sb")
                nc.sync.dma_start(w[:], t["wq_t"][h])
                wq_tiles[h] = w

            def q_chain(h, tag="pq", bufs=1):
                if h >= H:
                    return
                pq = pp2.tile([P, 512], F32, tag=tag, bufs=bufs, name="pq")
                w = wq_tiles.pop(h)
                for dc in range(DC):
                    nc.tensor.matmul(pq[:, :TQ], w[:, dc, :], hq[:, dc, :],
                                     start=(dc == 0), stop=(dc == DC - 1))
                rope2(pq[:, :TQ], cosq[:], sinq[:], QT[:, h, :])

            def av_chain(h, es1, es2):
                g = h // n_rep
                pav = pp2.tile([P, 512], F32, tag="pav", bufs=2, name="pav")
                for kc in range(8):
                    nc.tensor.matmul(
                        pav[:], Vt[:, kc, g * P:(g + 1) * P], es1[:, kc, :],
                        start=(kc == 0), stop=False,
                    )
                for m in range(8):
                    nc.tensor.matmul(
                        pav[:, 256:512], Vt[:, 8 + m, g * P:(g + 1) * P],
                        es2[:, m, :],
                        start=False, stop=(m == 7),
                    )
                return pav

            def den_chain_start(pes1, pes2):
                pden = pp2.tile([1, 512], F32, tag="small", bufs=1, name="pden")
                state = {"k": 0}

                def emit(n):
                    while n > 0 and state["k"] < 16:
                        k = state["k"]
                        if k < 8:
                            nc.tensor.matmul(pden[:], ones_bf[:], pes1[:, k, :],
                                             start=(k == 0), stop=False)
                        else:
                            nc.tensor.matmul(pden[:, 256:512], ones_bf[:],
                                             pes2[:, k - 8, :],
                                             start=False, stop=(k == 15))
                        state["k"] += 1
                        n -= 1
                return pden, emit

            def scores_den(h, prev):
                g = h // n_rep
                es1 = s2.tile([P, 8, 512], BF16, tag="es1", bufs=3, name="es1")
                es2 = s2.tile([P, 8, 256], BF16, tag="es2", bufs=3, name="es2")
                den_emit = None
                pden = None
                if prev is not None:
                    pden, den_emit = den_chain_start(*prev)
                for m in range(4):
                    pd = pp2.tile([P, 1024], F32, tag="dbl", bufs=2, name="pd")
                    for half in range(2):
                        kc = 2 * m + half
                        nc.tensor.matmul(
                            pd[:, half * 512:(half + 1) * 512],
                            KT[:, g, kc * P:(kc + 1) * P], QT[:, h, :],
                            start=True, stop=True,
                        )
                    if den_emit:
                        den_emit(3)
                    nc.scalar.activation(es1[:, 2 * m:2 * m + 2, :], pd[:],
                                         AF.Exp)
                    if m % 2 == 1:
                        nc.vector.tensor_tensor(
                            es1[:, 2 * m - 2:2 * m + 2, 0:256],
                            es1[:, 2 * m - 2:2 * m + 2, 0:256],
                            mk1[:, 2 * m - 2:2 * m + 2, :], OP.mult)
                for mq in range(2):
                    pd = pp2.tile([P, 1024], F32, tag="dbl", bufs=2, name="pdq")
                    for q4 in range(4):
                        kc = 8 + mq * 4 + q4
                        nc.tensor.matmul(
                            pd[:, q4 * 256:(q4 + 1) * 256],
                            KT[:, g, kc * P:(kc + 1) * P], QT[:, h, 256:512],
                            start=True, stop=True,
                        )
                    if den_emit:
                        den_emit(2)
                    nc.scalar.activation(es2[:, mq * 4:(mq + 1) * 4, :], pd[:],
                                         AF.Exp)
                    if mq == 1:
                        nc.vector.tensor_tensor(es2[:], es2[:], mk2[:],
                                                OP.mult)
                rden = None
                if pden is not None:
                    rden = misc.tile([1, 512], F32, tag="rden", bufs=2,
                                     name="rden")
                    recip(rden[:], pden[:])
                return es1, es2, rden

            def finish_head(ph, ppav, prden):
                pbc = pp2.tile([P, 512], F32, tag="pq", bufs=1, name="pbc")
                nc.tensor.matmul(pbc[:], ones32[:], prden[:], start=True,
                                 stop=True)
                rdba = s2.tile([P, 512], F32, tag="rdba", bufs=2, name="rdba")
                nc.scalar.activation(rdba[:], pbc[:], AF.Copy)
                nc.vector.tensor_tensor(attnT[:, ph, :], ppav[:], rdba[:],
                                        OP.mult)

            # 3-deep Q lead: the first two chains park in the (still unused)
            # pav ring so the pq ring's rope-read turnaround never serializes
            # them, and the PE has ~10us of work while the phase-1 DVE
            # backlog (K ropes) drains before scores(0) can start.
            wq_fetch(0)
            wq_fetch(1)
            wq_fetch(2)
            wq_fetch(3)
            q_chain(0, tag="pav", bufs=2)
            q_chain(1, tag="pav", bufs=2)
            q_chain(2)
            es = {}
            pav_d = {}
            rden_d = {}
            for h in range(H):
                if h >= 2:
                    finish_head(h - 2, pav_d.pop(h - 2), rden_d.pop(h - 2))
                if h >= 1:
                    pav_d[h - 1] = av_chain(h - 1, *es[h - 1])
                wq_fetch(h + 4)
                q_chain(h + 3)
                es1, es2, rden = scores_den(h, es.get(h - 1))
                es[h] = (es1, es2)
                if rden is not None:
                    rden_d[h - 1] = rden
            # tail: den(15), av(15), finish(14), finish(15)
            pes1, pes2 = es[H - 1]
            pdenT, den_emitT = den_chain_start(pes1, pes2)
            den_emitT(16)
            rdenT = misc.tile([1, 512], F32, tag="rden", bufs=2, name="rdenT")
            recip(rdenT[:], pdenT[:])
            pav_d[H - 1] = av_chain(H - 1, pes1, pes2)
            rden_d[H - 1] = rdenT
            finish_head(H - 2, pav_d.pop(H - 2), rden_d.pop(H - 2))
            finish_head(H - 1, pav_d.pop(H - 1), rden_d.pop(H - 1))

        p0b.release()   # hq/cos dead
        p_att.release()  # KT/Vt/QT dead

        # ============= phase 3: o-proj + residual + RMSNorm2 ===============
        p_res = ctx.enter_context(
            tc.tile_pool(name="p_res", bufs=1, side="right"))
        h2 = p_res.tile([P, DC, TQ], F32, tag="h2")
        mt = p_res.tile([P, DC, TQ], F16, tag="mt")
        rdb2 = p_res.tile([P, TQ], F32, tag="rdb2")
        # prefetch the first gate/up weight tiles into right-side staging so
        # phase 4's first chains don't wait on the pool transition
        wg0 = p_res.tile([P, DC, P], F16, tag="wg0")
        nc.sync.dma_start(wg0[:], t["wg_t"][0])
        wu0 = p_res.tile([P, DC, P], F16, tag="wu0")
        nc.sync.dma_start(wu0[:], t["wu_t"][0])
        pp3 = ctx.enter_context(tc.tile_pool(name="pp3", bufs=1, space="PSUM"))
        with tc.tile_pool(name="s3", bufs=1, side="left") as s3:
            var2 = pp3.tile([1, 512], F32, tag="small", bufs=1, name="var2")
            for dt in range(DC):
                wo_sb = s3.tile([P, H, P], F16, tag="wo", bufs=3)
                nc.sync.dma_start(wo_sb[:], t["wo_t"][dt])
                po = pp3.tile([P, 512], F32, tag="big", bufs=6, name="po")
                for ec in range(H):
                    nc.tensor.matmul(
                        po[:, :TQ], wo_sb[:, ec, :], attnT[:, ec, :],
                        start=(ec == 0), stop=(ec == H - 1),
                    )
                xqr = s3.tile([P, TQ], F32, tag="xq2", bufs=2)
                nc.sync.dma_start(xqr[:], t["xq32"][dt * P:(dt + 1) * P, :])
                nc.vector.tensor_tensor(h2[:, dt, :], po[:, :TQ], xqr[:], OP.add)
                nc.scalar.activation(mt[:, dt, :], h2[:, dt, :], AF.Copy)
                sq = s3.tile([P, TQ], F16, tag="sq3", bufs=3)
                nc.vector.tensor_tensor(sq[:], h2[:, dt, :], h2[:, dt, :],
                                        OP.mult)
                nc.tensor.matmul(var2[:, :TQ], ones16[:], sq[:],
                                 start=(dt == 0), stop=(dt == DC - 1))
            r2 = rstd_from_var(var2[:, :TQ], D)
            pbc2 = pp3.tile([P, 512], F32, tag="bc", bufs=1, name="pbc2")
            nc.tensor.matmul(pbc2[:, :TQ], ones32[:], r2[:], start=True, stop=True)
            nc.vector.tensor_copy(rdb2[:], pbc2[:, :TQ])
            # normalize mt once (mt = h2 * rstd2, fp16) so the MLP silu chain
            # needs no per-tile rstd folding (2 DVE ops/tile instead of 4)
            for dt in range(DC):
                nc.vector.tensor_tensor(mt[:, dt, :], mt[:, dt, :], rdb2[:],
                                        OP.mult)

        p_att2.release()  # attnT dead

        # ============= phase 4: MLP gate/up + silu =========================
        pp4 = pp3  # same PSUM pool/tags: no bank-transition stall
        with tc.tile_pool(name="p_gu", bufs=1, side="left") as p_gu, \
                tc.tile_pool(name="s45", bufs=1, side="left") as s4:
            gu = p_gu.tile([P, IT, TQ], F16, tag="gu")
            for it in range(IT):
                if it == 0:
                    wg_sb, wu_sb = wg0, wu0
                else:
                    wg_sb = s4.tile([P, DC, P], F16, tag="wgu", bufs=4)
                    nc.sync.dma_start(wg_sb[:], t["wg_t"][it])
                    wu_sb = s4.tile([P, DC, P], F16, tag="wgu", bufs=4)
                    nc.sync.dma_start(wu_sb[:], t["wu_t"][it])
                pg = pp4.tile([P, 512], F32, tag="big", bufs=7, name="pg")
                for dc in range(DC):
                    nc.tensor.matmul(pg[:, :TQ], wg_sb[:, dc, :],
                                     mt[:, dc, :],
                                     start=(dc == 0), stop=(dc == DC - 1))
                pu = pp4.tile([P, 512], F32, tag="big", bufs=7, name="pu")
                for dc in range(DC):
                    nc.tensor.matmul(pu[:, :TQ], wu_sb[:, dc, :],
                                     mt[:, dc, :],
                                     start=(dc == 0), stop=(dc == DC - 1))
                # mt is pre-normalized, so pg = g*r and pu = u*r directly:
                # gu = silu(g*r)*(u*r) = pg * sig(pg) * pu.
                sg = s4.tile([P, TQ], F16, tag="sg", bufs=3)
                nc.scalar.activation(sg[:], pg[:, :TQ], AF.Sigmoid)
                t2 = s4.tile([P, TQ], F16, tag="gg", bufs=3)
                nc.vector.tensor_tensor(t2[:], sg[:], pu[:, :TQ], OP.mult)
                nc.vector.tensor_tensor(gu[:, it, :], t2[:], pg[:, :TQ],
                                        OP.mult)

            # ============= phase 5: MLP down + residual ====================
            for dt in range(DC):
                wd_sb = s4.tile([P, IT, P], F16, tag="wd", bufs=2)
                nc.sync.dma_start(wd_sb[:], t["wd_t"][dt])
                pd5 = pp4.tile([P, 512], F32, tag="big", bufs=7,
                               name="pd5")
                for ic in range(IT):
                    nc.tensor.matmul(pd5[:, :TQ], wd_sb[:, ic, :],
                                     gu[:, ic, :],
                                     start=(ic == 0), stop=(ic == IT - 1))
                outp = s4.tile([P, TQ], F32, tag="out", bufs=3)
                nc.vector.tensor_tensor(outp[:], pd5[:, :TQ],
                                        h2[:, dt, :], OP.add)
                nc.sync.dma_start(t["outT"][dt * P:(dt + 1) * P, :],
                                  outp[:])


# --------------------------------------------------------------------------
# host-side input prep
# --------------------------------------------------------------------------

def _permute_heads(w, nheads):
    """Reorder each head's 128 rows as [even dims, odd dims] so RoPE's
    interleaved pairs become contiguous halves on-chip."""
    perm = np.concatenate([np.arange(0, P, 2), np.arange(1, P, 2)])
    return w.reshape(nheads, P, -1)[:, perm, :].reshape(nheads * P, -1)


def prep_weights(wq, wk, wv, wo, w_gate, w_up, w_down, ln1_w, ln2_w):
    f16 = np.float16
    c = np.ascontiguousarray

    wq_p = _permute_heads(wq * ln1_w[None, :], H)
    wk_p = _permute_heads(wk * ln1_w[None, :], KVH)
    wv_f = wv * ln1_w[None, :]
    wg_f = w_gate * ln2_w[None, :]
    wu_f = w_up * ln2_w[None, :]

    out = {}
    # lhsT tile layouts: [outer_tile, partition(128), inner_seq, free(128)]
    out["wq_t"] = c(wq_p.reshape(H, P, DC, P).transpose(0, 3, 2, 1).astype(f16))
    wk_l = wk_p.reshape(KVH, P, DC, P).transpose(0, 3, 2, 1)  # [et, d, dc, of]
    if USE_CC:
        out["wk_t"] = c(wk_l.astype(f16))
    else:
        out["wk_t2"] = c(wk_l.transpose(2, 1, 0, 3).astype(f16))
    out["wv_r"] = c(wv_f.T.reshape(DC, P, KVH * P).astype(f16))
    out["wo_t"] = c(wo.reshape(DC, P, H, P).transpose(0, 3, 2, 1).astype(f16))
    out["wg_t"] = c(wg_f.reshape(IT, P, DC, P).transpose(0, 3, 2, 1).astype(f16))
    out["wu_t"] = c(wu_f.reshape(IT, P, DC, P).transpose(0, 3, 2, 1).astype(f16))
    out["wd_t"] = c(w_down.reshape(DC, P, IT, P).transpose(0, 3, 2, 1).astype(f16))
    return out


def own_tokens(core):
    j = core % 4
    return np.concatenate([np.arange(256 * j, 256 * (j + 1)),
                           np.arange(256 * (7 - j), 256 * (8 - j))])


def prep_core_inputs(core, weights, hidden_states, cos, sin, attention_mask):
    b = core // 4
    tok = own_tokens(core)
    c = np.ascontiguousarray
    f32 = np.float32
    bf16 = ml_dtypes.bfloat16

    m = dict(weights)
    xT = hidden_states[b].T.astype(f32)          # [D, S]
    xo = xT[:, tok]                              # [D, 512]
    m["xq32"] = c(xo)
    m["xq16"] = c(xo.astype(np.float16))
    cos_o = cos[b][tok, :64].T.astype(f32)  # [64, 512]
    sin_o = sin[b][tok, :64].T.astype(f32)
    m["cos_q"] = c(cos_o * SCALE)
    m["sin_q"] = c(sin_o * SCALE)
    if USE_CC:
        m["cos_k"] = c(cos_o)
        m["sin_k"] = c(sin_o)
    else:
        m["xkv"] = c(xT.astype(np.float16))
        m["cos_k"] = c(cos[b][:, :64].T.astype(f32))
        m["sin_k"] = c(sin[b][:, :64].T.astype(f32))

    am = attention_mask[b, 0]  # [S, S] additive
    vis = am > NEG_THRESH      # [q, k] boolean visibility
    q1, q2 = tok[:256], tok[256:]
    # mask1[kl, kc, ql] = vis[q1[ql], 128*kc + kl]   for kc 0..7
    m1 = vis[np.ix_(q1, np.arange(0, 1024))]          # [256, 1024]
    m["mask1"] = c(m1.T.reshape(8, P, 256).transpose(1, 0, 2).astype(bf16))
    m2 = vis[np.ix_(q2, np.arange(1024, 2048))]       # [256, 1024]
    m["mask2"] = c(m2.T.reshape(8, P, 256).transpose(1, 0, 2).astype(bf16))
    return m


# --------------------------------------------------------------------------
# entry point
# --------------------------------------------------------------------------

_NC_CACHE = {}


def _get_nc():
    if "v2" not in _NC_CACHE:
        _NC_CACHE["v2"] = build_nc()
    return _NC_CACHE["v2"]


def kernel(hidden_states, cos, sin, attention_mask,
           wq, wk, wv, wo, w_gate, w_up, w_down, ln1_w, ln2_w):
    global LAST_EXEC_NS
    nc = _get_nc()

    weights = prep_weights(
        np.asarray(wq, np.float32), np.asarray(wk, np.float32),
        np.asarray(wv, np.float32), np.asarray(wo, np.float32),
        np.asarray(w_gate, np.float32), np.asarray(w_up, np.float32),
        np.asarray(w_down, np.float32),
        np.asarray(ln1_w, np.float32), np.asarray(ln2_w, np.float32),
    )
    hs = np.asarray(hidden_states, np.float32)
    cos = np.asarray(cos, np.float32)
    sin = np.asarray(sin, np.float32)
    am = np.asarray(attention_mask, np.float32)

    in_maps = [prep_core_inputs(c, weights, hs, cos, sin, am)
               for c in range(8)]

    trace = bool(int(os.environ.get("KERNEL_TRACE", "0")))
    trace_cores = None
    if trace and os.environ.get("KERNEL_TRACE_ALL"):
        trace_cores = list(range(8))
    res = run_bass_kernel_spmd(
        nc, in_maps, core_ids=list(range(8)), trace=trace,
        trace_cores=trace_cores,
        tmpdir=os.environ.get("KERNEL_TRACE_DIR") or None,
    )
    LAST_EXEC_NS = res.exec_time_ns

    out = np.empty((hs.shape[0], S, D), np.float32)
    for c in range(8):
        b = c // 4
        out[b, own_tokens(c), :] = res.results[c]["outT"].T
    return out
